# revision 23
# baseline (speedup 1.0000x reference)
"""Deformable PS-ROI Align (2-pass pooling + FC) on 8 TRN2 NeuronCores.

Strategy (ROI batch-parallel): 16 ROIs per core; featuremap and FC weights
replicated. Each pooling pass is a dense region/sample matmul driven by bulk
dma_gather (Q7 mlp library), with all gather indices precomputed on the host:

- Pass 1: sample positions depend only on `rois`. Host computes, per ROI, the
  exact bbox pixel list and folded weights W1[(row),(bin)] = Bh*Bw/cnt. The
  device gathers the region rows (one dma_gather per ROI, f32) and contracts
  them on the TensorEngine, landing pooled channels on partitions
  ([128c, 49b] x 2 halves) ready for the FC contraction.
- FC: 98 accumulating matmuls against the replicated weight; offsets scaled
  by roi size to give tx, ty.
- Pass 2: the host runs a numpy replica of pass1+FC to precompute per-sample
  gather row indices, corner floors (w0, h0), valid masks and 1/cnt. The
  device recomputes only the *continuous* part (dw = clip(pos) - w0_host)
  from its own FC output, so sub-ulp host/device FC drift cannot flip any
  discrete decision; weights stay exact by linear extrapolation. One
  dma_gather per ROI-pair (bf16) fetches all 49 bins x 128 sample-corner
  rows (row-stream partition p = corner*32 + nl*16 + sh*4 + sw); per-bin
  matmuls reduce them with the parity-masked weights.

ROIs are dealt to (core, slot) sorted by region size so the per-slot k-tile
count (a compile-time constant shared by all cores) has minimal padding.
"""
import sys
import numpy as np

sys.path.insert(0, '/opt/trn_rl_repo')

from ml_dtypes import bfloat16

POOLED = 7
SAMPLE = 4
SCALE = np.float32(1.0 / 16.0)
B, H, W, C = 2, 128, 128, 256
N = 128
NCORES = 8
R = N // NCORES            # 16 rois per core
NPAIR = R // 2             # 8 roi-pairs per core
NBIN = POOLED * POOLED     # 49
NF = NPAIR * NBIN          # 392
NROWS = B * H * W          # 32768 feature pixels
F32 = np.float32

_COMPILED = {}             # KTs tuple -> compiled program
LAST_RESULTS = None


# ---------------------------------------------------------------- host math
def _roi_scalars(rois):
    r = rois.astype(F32)
    bidx = r[:, 0].astype(np.int32)
    x1 = np.round(r[:, 1]) * SCALE - F32(0.5)
    y1 = np.round(r[:, 2]) * SCALE - F32(0.5)
    x2 = (np.round(r[:, 3]) + F32(1.0)) * SCALE - F32(0.5)
    y2 = (np.round(r[:, 4]) + F32(1.0)) * SCALE - F32(0.5)
    rw = np.maximum(x2 - x1, F32(0.1))
    rh = np.maximum(y2 - y1, F32(0.1))
    return bidx, x1, y1, rw, rh


def _axis_tables(pos, limit):
    valid = ((pos >= F32(-0.5)) & (pos <= F32(limit - 0.5))).astype(F32)
    pc = np.clip(pos, F32(0.0), F32(limit - 1.0))
    i0 = np.floor(pc).astype(np.int32)
    i1 = np.minimum(i0 + 1, limit - 1)
    d = (pc - i0).astype(F32)
    return i0, i1, d, valid


def _sample_coords(rois, trans):
    """w[n,i,j,sw], h[n,i,j,sh] sample positions (f32, pre-clip)."""
    nroi = rois.shape[0]
    bidx, x1, y1, rw, rh = _roi_scalars(rois)
    bw = rw / F32(POOLED); bh = rh / F32(POOLED)
    sbw = bw / F32(SAMPLE); sbh = bh / F32(SAMPLE)
    pidx = np.arange(POOLED, dtype=F32)
    if trans is None:
        tx = np.zeros((nroi, POOLED, POOLED), F32)
        ty = np.zeros((nroi, POOLED, POOLED), F32)
    else:
        tx = trans[:, 0].astype(F32); ty = trans[:, 1].astype(F32)
    wstart = (pidx[None, None, :] * bw[:, None, None] + x1[:, None, None]
              + tx * rw[:, None, None]).astype(F32)
    hstart = (pidx[None, :, None] * bh[:, None, None] + y1[:, None, None]
              + ty * rh[:, None, None]).astype(F32)
    s = np.arange(SAMPLE, dtype=F32)
    w = (wstart[..., None] + s * sbw[:, None, None, None]).astype(F32)
    h = (hstart[..., None] + s * sbh[:, None, None, None]).astype(F32)
    return w, h


def _pass1_geometry(rois):
    """Per-ROI pass-1 separable axis weights + bbox."""
    bidx = rois[:, 0].astype(np.int32)
    w, h = _sample_coords(rois, None)        # [N,7,7,4]; axis-independent
    wj = w[:, 0, :, :]                       # [N,7j,4sw] (same for all i)
    hi = h[:, :, 0, :]                       # [N,7i,4sh]
    w0, w1i, dw, vw = _axis_tables(wj, W)
    h0, h1i, dh, vh = _axis_tables(hi, H)
    n_i = np.arange(N)[:, None, None]
    j_i = np.arange(POOLED)[None, :, None]
    Bw = np.zeros((N, POOLED, W), F32)
    Bh = np.zeros((N, POOLED, H), F32)
    np.add.at(Bw, (n_i, j_i, w0), (1 - dw) * vw)
    np.add.at(Bw, (n_i, j_i, w1i), dw * vw)
    np.add.at(Bh, (n_i, j_i, h0), (1 - dh) * vh)
    np.add.at(Bh, (n_i, j_i, h1i), dh * vh)
    cnt = (vh.sum(-1)[:, :, None] * vw.sum(-1)[:, None, :]).astype(F32)
    scale = np.where(cnt > 0, 1.0 / np.maximum(cnt, 1.0), 0.0).astype(F32)
    xmin = w0.min(axis=(1, 2)); xmax = w1i.max(axis=(1, 2))
    ymin = h0.min(axis=(1, 2)); ymax = h1i.max(axis=(1, 2))
    return dict(bidx=bidx, Bw=Bw, Bh=Bh, scale=scale,
                xmin=xmin, xmax=xmax, ymin=ymin, ymax=ymax)


def _assign(rois):
    """Deal ROIs (sorted by region size) to cores. perm[c,s] = roi id."""
    g = _pass1_geometry(rois)
    nrows = (g["ymax"] - g["ymin"] + 1) * (g["xmax"] - g["xmin"] + 1)
    order = np.argsort(-nrows, kind="stable")
    perm = np.empty((NCORES, R), np.int64)
    for k, n in enumerate(order):
        perm[k % NCORES, k // NCORES] = n
    kt = -(-nrows // 128)
    KTs = tuple(int(kt[perm[:, s]].max()) for s in range(R))
    return g, perm, KTs


def _wrap_idx16(flat_idx):
    """[n] row indices -> wrapped int16 [128, n//16] (n % 16 == 0)."""
    ncol = len(flat_idx) // 16
    blk = flat_idx.reshape(ncol, 16).T.astype(np.int16)      # [16, ncol]
    return np.broadcast_to(blk[None], (8, 16, ncol)).reshape(128, ncol)


def _pass1_tables(g, perm, KTs):
    """Per-core: idx1 int16 wrapped [128, 8*SKT], w1 f32 [128, SKT*49],
    plus per-(c,s) row lists (for the host pooling replica)."""
    SKT = sum(KTs)
    idx1 = np.zeros((NCORES, 128, 8 * SKT), np.int16)
    w1 = np.zeros((NCORES, 128, SKT, NBIN), F32)
    rowlists = {}
    for c in range(NCORES):
        off = 0
        for s in range(R):
            n = perm[c, s]
            KT = KTs[s]
            ym, yM = g["ymin"][n], g["ymax"][n]
            xm, xM = g["xmin"][n], g["xmax"][n]
            Hr, Wr = yM - ym + 1, xM - xm + 1
            nr = Hr * Wr
            ys, xs = np.divmod(np.arange(nr), Wr)
            ridx = (g["bidx"][n] * (H * W) + (ym + ys) * W + (xm + xs))
            Wv = np.einsum("iy,jx->yxij", g["Bh"][n][:, ym:yM + 1],
                           g["Bw"][n][:, xm:xM + 1]).reshape(nr, NBIN)
            Wv = (Wv * g["scale"][n].reshape(1, NBIN)).astype(F32)
            rowlists[(c, s)] = (ridx, Wv)
            pad = KT * 128 - nr
            ridxp = np.pad(ridx, (0, pad))
            Wvp = np.pad(Wv, ((0, pad), (0, 0)))
            idx1[c, :, off * 8:(off + KT) * 8] = _wrap_idx16(ridxp)
            w1[c, :, off:off + KT] = Wvp.reshape(KT, 128, NBIN).transpose(1, 0, 2)
            off += KT
    return (idx1,
            np.ascontiguousarray(w1.reshape(NCORES, 128, SKT * NBIN)),
            rowlists)


def _host_replica(featuremap, fc_w, fc_b, g, perm, rowlists):
    """Host pass-1 pooling (same W1 weights) + FC -> trans [N,2,7,7]."""
    fmr = featuremap.reshape(NROWS, C)
    pooled = np.zeros((N, NBIN, C), F32)
    for (c, s), (ridx, Wv) in rowlists.items():
        n = perm[c, s]
        pooled[n] = Wv.T @ fmr[ridx]
    off = (pooled.reshape(N, NBIN * C) @ fc_w + fc_b).astype(F32)
    return off.reshape(N, 2, POOLED, POOLED)


def _p_decode():
    p = np.arange(128)
    return p // 32, (p // 16) % 2, (p % 16) // 4, p % 4   # corner, nl, sh, sw


def _pass2_tables(rois_c, trans_c):
    """Per-core pass-2 tables in the row-stream layout.
    Returns dict with idx2 (int16 wrapped per pair), w0h/h0h/vm and the
    device position tables wb/hb/ssbw/ssbh, sel, par."""
    bidx, x1, y1, rw, rh = _roi_scalars(rois_c)
    bw = rw / F32(POOLED); bh = rh / F32(POOLED)
    sbw = bw / F32(SAMPLE); sbh = bh / F32(SAMPLE)
    corner, nl, sh, sw = _p_decode()
    bins = np.arange(NBIN)
    jb49 = (bins % POOLED).astype(F32)
    ib49 = (bins // POOLED).astype(F32)
    n_of = (2 * np.arange(NPAIR)[None, :] + nl[:, None])          # [128, 8]
    wb = (jb49[None, None, :] * bw[n_of][..., None]
          + x1[n_of][..., None]).astype(F32).reshape(128, NF)
    hb = (ib49[None, None, :] * bh[n_of][..., None]
          + y1[n_of][..., None]).astype(F32).reshape(128, NF)
    ssbw = np.broadcast_to((sw[:, None].astype(F32) * sbw[n_of])[..., None],
                           (128, NPAIR, NBIN)).astype(F32).reshape(128, NF)
    ssbh = np.broadcast_to((sh[:, None].astype(F32) * sbh[n_of])[..., None],
                           (128, NPAIR, NBIN)).astype(F32).reshape(128, NF)

    # host sampling for this core's rois with its trans
    w, h = _sample_coords(rois_c, trans_c)        # [R,7,7,4]
    w0, w1i, _dw, vw = _axis_tables(w, W)
    h0, h1i, _dh, vh = _axis_tables(h, H)
    cnt = (vh.sum(-1) * vw.sum(-1)).astype(F32)   # [R,7,7]
    sc2 = np.where(cnt > 0, 1.0 / np.maximum(cnt, 1.0), 0.0).astype(F32)

    ii = (bins // POOLED)[None, :]
    jj = (bins % POOLED)[None, :]
    nn = n_of.reshape(128, NPAIR, 1)
    iif = np.broadcast_to(ii[None], (128, NPAIR, NBIN))
    jjf = np.broadcast_to(jj[None], (128, NPAIR, NBIN))
    shf = np.broadcast_to(sh[:, None, None], (128, NPAIR, NBIN))
    swf = np.broadcast_to(sw[:, None, None], (128, NPAIR, NBIN))
    cf = corner[:, None, None]
    hsel = np.where(cf < 2, h0[nn, iif, jjf, shf], h1i[nn, iif, jjf, shf])
    wsel = np.where(cf % 2 == 0, w0[nn, iif, jjf, swf], w1i[nn, iif, jjf, swf])
    rows = (bidx[nn] * (H * W) + hsel * W + wsel).reshape(128, NPAIR, NBIN)
    w0h = w0[nn, iif, jjf, swf].astype(F32).reshape(128, NF)
    h0h = h0[nn, iif, jjf, shf].astype(F32).reshape(128, NF)
    vm = (vh[nn, iif, jjf, shf] * vw[nn, iif, jjf, swf]
          * sc2[nn, iif, jjf]).astype(F32).reshape(128, NF)

    # wrapped idx per pair: k = bin*128 + p; 6272/16 = 392 cols per pair
    idx2 = np.zeros((128, NPAIR * NF), np.int16)
    for pr in range(NPAIR):
        flat = rows[:, pr, :].T.reshape(-1)            # k = bin*128 + p
        idx2[:, pr * NF:(pr + 1) * NF] = _wrap_idx16(flat)

    sel = np.zeros((R, NPAIR, 128), F32)
    for pr in range(NPAIR):
        sel[2 * pr + nl, pr, np.arange(128)] = 1.0
    sel = np.ascontiguousarray(sel.reshape(R, NPAIR * 128))
    par = np.zeros((128, 2 * NBIN), F32)
    for p in range(128):
        par[p, nl[p]::2] = 1.0
    rwrh = np.zeros((R, 2 * NBIN), F32)
    rwrh[:, :NBIN] = rw[:, None]
    rwrh[:, NBIN:] = rh[:, None]
    return dict(wb128=wb, hb128=hb, ssbw=ssbw, ssbh=ssbh,
                w0h=w0h, h0h=h0h, vm=vm, idx2=idx2,
                sel=sel, par=par, rwrh16=rwrh)


# ---------------------------------------------------------------- program
def _build_program(KTs):
    import concourse.bass as bass
    import concourse.bacc as bacc
    import concourse.tile as tile
    import concourse.mybir as mybir
    from concourse.ap import AP
    from concourse import library_config

    f32, i16 = mybir.dt.float32, mybir.dt.int16
    bf16 = mybir.dt.bfloat16
    A = mybir.AluOpType
    SKT = sum(KTs)
    KTmax = max(KTs)

    def rap(apx, dims, extra_offset=0):
        return AP(tensor=apx.tensor, offset=apx.offset + extra_offset,
                  ap=[list(apx.ap[0])] + [list(d) for d in dims])

    nc = bacc.Bacc("TRN2", target_bir_lowering=False, debug=False,
                   num_devices=NCORES, num_swdge_queues=2)
    feat_d = nc.dram_tensor("feat", [NROWS, C], f32, kind="ExternalInput").ap()
    fm16_d = nc.dram_tensor("fm16", [NROWS, C], bf16, kind="ExternalInput").ap()
    idx1_d = nc.dram_tensor("idx1", [128, 8 * SKT], i16, kind="ExternalInput").ap()
    w1_d = nc.dram_tensor("w1", [128, SKT * NBIN], f32, kind="ExternalInput").ap()
    idx2_d = nc.dram_tensor("idx2", [128, NPAIR * NF], i16,
                            kind="ExternalInput").ap()
    fcw_d = nc.dram_tensor("fcw", [128, 98 * 98], f32, kind="ExternalInput").ap()
    wb_d = nc.dram_tensor("wb128", [128, NF], f32, kind="ExternalInput").ap()
    hb_d = nc.dram_tensor("hb128", [128, NF], f32, kind="ExternalInput").ap()
    ssbw_d = nc.dram_tensor("ssbw", [128, NF], f32, kind="ExternalInput").ap()
    ssbh_d = nc.dram_tensor("ssbh", [128, NF], f32, kind="ExternalInput").ap()
    w0h_d = nc.dram_tensor("w0h", [128, NF], f32, kind="ExternalInput").ap()
    h0h_d = nc.dram_tensor("h0h", [128, NF], f32, kind="ExternalInput").ap()
    vm_d = nc.dram_tensor("vm", [128, NF], f32, kind="ExternalInput").ap()
    rwrh_d = nc.dram_tensor("rwrh16", [R, 98], f32, kind="ExternalInput").ap()
    fcb_d = nc.dram_tensor("fcb16", [R, 98], f32, kind="ExternalInput").ap()
    sel_d = nc.dram_tensor("sel", [R, NPAIR * 128], f32, kind="ExternalInput").ap()
    par_d = nc.dram_tensor("par", [128, 2 * NBIN], f32, kind="ExternalInput").ap()
    out_d = nc.dram_tensor("out", [NPAIR, 2, 128, 2 * NBIN], f32,
                           kind="ExternalOutput").ap()

    with tile.TileContext(nc) as tc:
        with (tc.tile_pool(name="const", bufs=1) as cp,
              tc.tile_pool(name="gath1", bufs=2) as gp1,
              tc.tile_pool(name="gath2", bufs=3) as gp2,
              tc.tile_pool(name="work", bufs=1) as wp,
              tc.tile_pool(name="ph3", bufs=1) as p3,
              tc.tile_pool(name="psA", bufs=2, space="PSUM") as psA,
              tc.tile_pool(name="psB", bufs=1, space="PSUM") as psB):
            nc.gpsimd.load_library(library_config.mlp)
            # ---------- preload ----------
            idx1_s = cp.tile([128, 8 * SKT], i16)
            nc.sync.dma_start(idx1_s[:], idx1_d)
            w1_s = cp.tile([128, SKT * NBIN], f32)
            nc.sync.dma_start(w1_s[:], w1_d)
            idx2_s = cp.tile([128, NPAIR * NF], i16)
            nc.sync.dma_start(idx2_s[:], idx2_d)

            flatT3 = wp.tile([128, 98, R], f32)   # [c, q=(b*2+h), r]

            # ---------- pass 1 ----------
            off = 0
            for s in range(R):
                KT = KTs[s]
                g = gp1.tile([128, KTmax, C], f32, tag="g1")
                nc.gpsimd.dma_gather(g[:, 0:KT, :], feat_d,
                                     idx1_s[:, off * 8:(off + KT) * 8],
                                     KT * 128, KT * 128, C, queue_num=s % 2)
                pslo = psA.tile([128, 2 * NBIN], f32, tag="plo")
                pshi = psA.tile([128, 2 * NBIN], f32, tag="phi")
                for t in range(KT):
                    wcols = w1_s[:, (off + t) * NBIN:(off + t + 1) * NBIN]
                    nc.tensor.matmul(pslo[:, 0:NBIN], g[:, t, 0:128], wcols,
                                     start=(t == 0), stop=(t == KT - 1))
                for t in range(KT):
                    wcols = w1_s[:, (off + t) * NBIN:(off + t + 1) * NBIN]
                    nc.tensor.matmul(pshi[:, 0:NBIN], g[:, t, 128:256], wcols,
                                     start=(t == 0), stop=(t == KT - 1))
                nc.vector.tensor_copy(
                    rap(flatT3[:], [[2 * R, NBIN]], extra_offset=s),
                    pslo[:, 0:NBIN])
                nc.scalar.copy(
                    rap(flatT3[:], [[2 * R, NBIN]], extra_offset=R + s),
                    pshi[:, 0:NBIN])
                off += KT

            # remaining consts queue behind the pass-1 gather DMAs
            fcw_s = cp.tile([128, 98 * 98], f32)
            nc.sync.dma_start(fcw_s[:], fcw_d)
            wb_s = cp.tile([128, NF], f32); nc.sync.dma_start(wb_s[:], wb_d)
            hb_s = cp.tile([128, NF], f32); nc.sync.dma_start(hb_s[:], hb_d)
            ssbw_s = cp.tile([128, NF], f32); nc.sync.dma_start(ssbw_s[:], ssbw_d)
            ssbh_s = cp.tile([128, NF], f32); nc.sync.dma_start(ssbh_s[:], ssbh_d)
            w0h_s = cp.tile([128, NF], f32); nc.sync.dma_start(w0h_s[:], w0h_d)
            h0h_s = cp.tile([128, NF], f32); nc.sync.dma_start(h0h_s[:], h0h_d)
            vm_s = cp.tile([128, NF], f32); nc.sync.dma_start(vm_s[:], vm_d)
            rwrh_s = cp.tile([R, 98], f32); nc.sync.dma_start(rwrh_s[:], rwrh_d)
            fcb_s = cp.tile([R, 98], f32); nc.sync.dma_start(fcb_s[:], fcb_d)
            sel_s = cp.tile([R, NPAIR * 128], f32); nc.sync.dma_start(sel_s[:], sel_d)
            par_s = cp.tile([128, 2 * NBIN], f32); nc.sync.dma_start(par_s[:], par_d)

            # ---------- FC ----------
            fc_ps = psB.tile([R, 98], f32, tag="fc")
            for q in range(98):
                nc.tensor.matmul(fc_ps[:], flatT3[:, q, :],
                                 fcw_s[:, q * 98:(q + 1) * 98],
                                 start=(q == 0), stop=(q == 97))
            off_s = wp.tile([R, 98], f32)
            nc.vector.tensor_tensor(off_s[:], fc_ps[:], fcb_s[:], op=A.add)
            txys = wp.tile([R, 98], f32)
            nc.vector.tensor_tensor(txys[:], off_s[:], rwrh_s[:], op=A.mult)

            # ---------- broadcast tx, ty ----------
            txb_ps = psB.tile([128, NF], f32, tag="txb")
            tyb_ps = psB.tile([128, NF], f32, tag="tyb")
            for pr in range(NPAIR):
                lhs = sel_s[:, pr * 128:(pr + 1) * 128]
                nc.tensor.matmul(txb_ps[:, pr * NBIN:(pr + 1) * NBIN], lhs,
                                 txys[:, 0:49], start=True, stop=True)
                nc.tensor.matmul(tyb_ps[:, pr * NBIN:(pr + 1) * NBIN], lhs,
                                 txys[:, 49:98], start=True, stop=True)

            # ---------- pass-2 weights (continuous part only) ----------
            def ts(out, in0, s1, s2, o0, o1=None):
                if o1 is None:
                    nc.vector.tensor_scalar(out, in0, s1, None, op0=o0)
                else:
                    nc.vector.tensor_scalar(out, in0, s1, s2, op0=o0, op1=o1)

            posw = p3.tile([128, NF], f32, tag="tposw")
            nc.vector.tensor_tensor(posw[:], wb_s[:], txb_ps[:], op=A.add)
            nc.vector.tensor_tensor(posw[:], posw[:], ssbw_s[:], op=A.add)
            posh = p3.tile([128, NF], f32, tag="tposh")
            nc.vector.tensor_tensor(posh[:], hb_s[:], tyb_ps[:], op=A.add)
            nc.vector.tensor_tensor(posh[:], posh[:], ssbh_s[:], op=A.add)
            ts(posw[:], posw[:], 0.0, float(W - 1), A.max, A.min)   # wc
            ts(posh[:], posh[:], 0.0, float(H - 1), A.max, A.min)   # hc
            dwt = p3.tile([128, NF], f32, tag="tdwt")
            nc.vector.tensor_tensor(dwt[:], posw[:], w0h_s[:], op=A.subtract)
            dht = p3.tile([128, NF], f32, tag="tdht")
            nc.vector.tensor_tensor(dht[:], posh[:], h0h_s[:], op=A.subtract)
            omdw = p3.tile([128, NF], f32, tag="tomdw")
            ts(omdw[:], dwt[:], -1.0, 1.0, A.mult, A.add)
            omdh = p3.tile([128, NF], f32, tag="tomdh")
            ts(omdh[:], dht[:], -1.0, 1.0, A.mult, A.add)
            wgt = p3.tile([128, NF], f32, tag="twgt")
            for c_ in range(4):
                sl = slice(c_ * 32, c_ * 32 + 32)
                hsel = (omdh if c_ < 2 else dht)
                wsel = (omdw if c_ % 2 == 0 else dwt)
                nc.vector.tensor_tensor(wgt[sl, :], hsel[sl, :], wsel[sl, :],
                                        op=A.mult)
                nc.vector.tensor_tensor(wgt[sl, :], wgt[sl, :], vm_s[sl, :],
                                        op=A.mult)

            # ---------- pass 2 (per pair) ----------
            for pr in range(NPAIR):
                lh2s = wp.tile([128, 2 * NBIN], bf16, tag="lh2s")
                wsrc = rap(wgt[:], [[1, NBIN], [0, 2]], extra_offset=pr * NBIN)
                nc.vector.tensor_tensor(lh2s[:], wsrc, par_s[:], op=A.mult)

                g2 = gp2.tile([128, NBIN, C], bf16, tag="g2")
                # SWDGE ring holds 1024 descriptors; gather 7 bins (896) a
                # call, alternating queues so gen(n+1) overlaps transfer(n)
                for ci, b0 in enumerate(range(0, NBIN, 7)):
                    nb = min(7, NBIN - b0)
                    nc.gpsimd.dma_gather(
                        g2[:, b0:b0 + nb, :], fm16_d,
                        idx2_s[:, pr * NF + b0 * 8:pr * NF + (b0 + nb) * 8],
                        nb * 128, nb * 128, C, queue_num=(pr * 7 + ci) % 2)
                p2lo = psA.tile([128, 2 * NBIN], f32, tag="plo")
                p2hi = psA.tile([128, 2 * NBIN], f32, tag="phi")
                for b in range(NBIN):
                    lhcols = lh2s[:, 2 * b:2 * b + 2]
                    nc.tensor.matmul(p2lo[:, 2 * b:2 * b + 2], g2[:, b, 0:128],
                                     lhcols, start=True, stop=True)
                    nc.tensor.matmul(p2hi[:, 2 * b:2 * b + 2], g2[:, b, 128:256],
                                     lhcols, start=True, stop=True)
                s2lo = wp.tile([128, 2 * NBIN], f32, tag="s2lo")
                nc.vector.tensor_copy(s2lo[:], p2lo[:])
                s2hi = wp.tile([128, 2 * NBIN], f32, tag="s2hi")
                nc.scalar.copy(s2hi[:], p2hi[:])
                nc.sync.dma_start(out_d[pr, 0], s2lo[:])
                nc.sync.dma_start(out_d[pr, 1], s2hi[:])

    nc.compile()
    return nc


def _get_compiled(KTs):
    if KTs not in _COMPILED:
        _COMPILED[KTs] = _build_program(KTs)
    return _COMPILED[KTs]


def kernel(featuremap, rois, fc_w, fc_b):
    global LAST_RESULTS
    from concourse.bass_utils import run_bass_kernel_spmd

    featuremap = np.ascontiguousarray(featuremap, dtype=np.float32)
    rois = np.ascontiguousarray(rois, dtype=np.float32)
    fc_w = np.ascontiguousarray(fc_w, dtype=np.float32)
    fc_b = np.ascontiguousarray(fc_b, dtype=np.float32)

    g, perm, KTs = _assign(rois)
    nc = _get_compiled(KTs)

    fmr = featuremap.reshape(NROWS, C)
    fm16 = fmr.astype(bfloat16)
    idx1, w1, rowlists = _pass1_tables(g, perm, KTs)
    trans = _host_replica(featuremap, fc_w, fc_b, g, perm, rowlists)
    fcw_r = np.ascontiguousarray(
        fc_w.reshape(NBIN, 2, 128, 98).transpose(2, 0, 1, 3)
    ).reshape(128, 98 * 98)
    fcb16 = np.broadcast_to(fc_b.astype(F32)[None, :], (R, 98)).copy()

    maps = []
    for c in range(NCORES):
        m = _pass2_tables(rois[perm[c]], trans[perm[c]])
        m["feat"] = fmr
        m["fm16"] = fm16
        m["idx1"] = idx1[c]
        m["w1"] = w1[c]
        m["fcw"] = fcw_r
        m["fcb16"] = fcb16
        maps.append(m)

    res = run_bass_kernel_spmd(nc, maps, core_ids=list(range(NCORES)))
    LAST_RESULTS = res
    out = np.empty((N, POOLED, POOLED, C), np.float32)
    for c in range(NCORES):
        o = res.results[c]["out"]          # [pair, half, 128c, 2b+nl]
        for pr in range(NPAIR):
            for nl_ in range(2):
                r_out = o[pr, :, :, nl_::2]                      # [2, 128, 49]
                full = np.concatenate([r_out[0], r_out[1]], axis=0)  # [256, 49]
                out[perm[c, 2 * pr + nl_]] = full.T.reshape(
                    POOLED, POOLED, C)
    return out
